# revision 9
# baseline (speedup 1.0000x reference)
"""EnhancedCorrelationGNN Trainium2 kernel (8 NeuronCores, SPMD), v2.

Strategy: destination-sorted edge processing, node-range output sharding,
bf16 edge pipeline, no collectives.
 - Host (free): counting-sort edges by dst, partition nodes into 8 ranges of
   6272 (49 blocks x 128 nodes per core). Per block, edges are split by src
   half (dma_gather int16 index limit) and padded to 128-edge tiles with
   cross-core-uniform tile counts (one SPMD program). Host also ships, per
   chunk, the one-hot scatter matrices S (edge->dst, agg lhsT) and S2
   (dst->edge, ad-apply lhsT) as fp8 bytes, plus c = ew*epw+epb in bf16.
 - Phase 1 (device, REPLICATED): every core computes the FULL node table
   h|as from x (bf16), writing 512B rows [h_dmaj(128)|as(8)|pad] to its own
   DRAM - no AllGather, no cross-core sync at all. ad for the core's own
   node range is computed from x_t_own and kept in SBUF.
 - Phase 2 (device): per 16-tile chunk, ONE dma_gather of 512B bf16 rows by
   src (the <512B descriptor penalty makes a separate ad gather cost as much
   as this one - hence S2-matmul instead), one u8 DMA of [S|S2];
   per tile a tiny matmul S2^T@ad_blk gives per-edge ad; scores via
   DVE add -> ScalarE Lrelu -> DVE add c -> ScalarE Exp (p in bf16);
   msgs = h*p on DVE in 2x mode (d-major h layout keeps last dim packed);
   per-tile agg matmul (lhsT=S fp8, rhs=[msgs|p] bf16) accumulates
   [sum msgs | sum p] per block; normalize by 1/(sum p+1e-10), add bias.
"""
import sys

if "/opt/trn_rl_repo" not in sys.path:
    sys.path.insert(0, "/opt/trn_rl_repo")

import numpy as np

import concourse.bass as bass
import concourse.bacc as bacc
import concourse.mybir as mybir
import concourse.tile as tile
from concourse.bass_utils import run_bass_kernel_spmd

# ---------------------------------------------------------------- constants
N = 50000
E = 800000
IN_F = 128
H = 8
HD = 16
OUT_F = H * HD          # 128
ALPHA = 0.2
EPS = 1e-10

NCORES = 8
P = 128
NPC = 6272              # nodes per core = 49 * 128; 8*6272 = 50176 >= N
NPAD = NCORES * NPC     # 50176
NBLK = NPC // P         # 49
HALF = NPAD // 2        # 25088 (int16 gather index limit)

ROWE = 256              # hs table row: bf16 elems [h_dmaj(128)|as(8)|pad]
AS_OFF = 128
CHUNK_TILES = 16        # tiles per gather/DVE chunk
IDX_COLS = CHUNK_TILES * P // 16   # wrapped int16 idx columns per chunk
PAD_DSTL = 300.0        # one-hot miss sentinel
GRP = 3                 # phase-1 matmul tiles batched per PSUM copy

FP = mybir.dt.float32
BF = mybir.dt.bfloat16
U8 = mybir.dt.uint8
FP8 = mybir.dt.float8e4
ONE_FP8 = 0x38          # 1.0 in e4m3

# build stages for HW bisection: 1=phase1 only, 2=+gathers+S loads,
# 3=+chunk compute pipeline, 4=full (default)
BUILD_STAGE = 4
# repeat whole kernel body inside one NEFF (for timing by differencing)
REPS = 1


# ---------------------------------------------------------------- planning
def _cdiv(a, b):
    return -(-a // b)


def _wrap_idx(idx_flat: np.ndarray) -> np.ndarray:
    """[n] -> [128, IDX_COLS] int16: idx j at [j%16, j//16], replicated x8."""
    n = idx_flat.shape[0]
    assert n % 16 == 0
    w = idx_flat.reshape(n // 16, 16).T.astype(np.int16)      # [16, n/16]
    w = np.tile(w, (8, 1))                                    # [128, n/16]
    out = np.zeros((P, IDX_COLS), dtype=np.int16)
    out[:, : w.shape[1]] = w
    return out


def plan_and_inputs(edge_index, edge_weight):
    """Host-side edge partitioning. Returns (plan, per_core_arrays).

    plan (core-independent, defines the SPMD program):
      KA, KB: [NBLK] tiles per (block, half)
      chunks: list of dicts(stream, g0, nt) over stream-major tile ids
      block_tiles: per block, list of (chunk_id, slot) in matmul order
      tile_block: [T] block index of each tile
      T, T_A, n_chunks
    per_core_arrays[c]:
      src_idx [n_chunks,128,IDX_COLS] i16 (half-relative)
      s2s     [n_chunks,128,CHUNK_TILES*256] u8 (fp8 bytes: per tile
              cols 0:128 = S[e_part, d], cols 128:256 = S2[d_part, e])
      ew      [T, 128] f32 (slot edge weights, for c = ew*epw+epb)
    """
    src = np.asarray(edge_index[0], dtype=np.int64)
    dst = np.asarray(edge_index[1], dtype=np.int64)
    ew = np.asarray(edge_weight, dtype=np.float32)

    order = np.argsort(dst, kind="stable")
    src_s, dst_s, ew_s = src[order], dst[order], ew[order]

    cnt = np.zeros((NCORES, NBLK, 2), dtype=np.int64)
    lists = [[[None, None] for _ in range(NBLK)] for _ in range(NCORES)]
    blk_starts = np.searchsorted(dst_s, np.arange(0, NPAD + 1, P))
    for c in range(NCORES):
        for b in range(NBLK):
            g = c * NBLK + b
            lo, hi = blk_starts[g], blk_starts[g + 1]
            s = src_s[lo:hi]
            mA = s < HALF
            idxs = np.arange(lo, hi)
            lists[c][b][0] = idxs[mA]
            lists[c][b][1] = idxs[~mA]
            cnt[c, b, 0] = mA.sum()
            cnt[c, b, 1] = (~mA).sum()

    KA = np.maximum(_cdiv(cnt[:, :, 0].max(axis=0), P), 1).astype(np.int64)
    KB = _cdiv(cnt[:, :, 1].max(axis=0), P).astype(np.int64)

    T_A = int(KA.sum())
    T_B = int(KB.sum())
    T = T_A + T_B
    cumKA = np.concatenate([[0], np.cumsum(KA)])
    cumKB = np.concatenate([[0], np.cumsum(KB)])

    chunks = []
    g = 0
    while g < T_A:
        nt = min(CHUNK_TILES, T_A - g)
        chunks.append(dict(stream=0, g0=g, nt=nt))
        g += nt
    while g < T:
        nt = min(CHUNK_TILES, T - g)
        chunks.append(dict(stream=1, g0=g, nt=nt))
        g += nt
    n_chunks = len(chunks)

    chunk_of = np.empty(T, dtype=np.int64)
    slot_of = np.empty(T, dtype=np.int64)
    for ci, ch in enumerate(chunks):
        chunk_of[ch["g0"]: ch["g0"] + ch["nt"]] = ci
        slot_of[ch["g0"]: ch["g0"] + ch["nt"]] = np.arange(ch["nt"])

    tile_block = np.empty(T, dtype=np.int64)
    block_tiles = []
    for b in range(NBLK):
        tl = []
        for k in range(KA[b]):
            gidx = cumKA[b] + k
            tl.append((int(chunk_of[gidx]), int(slot_of[gidx])))
            tile_block[gidx] = b
        for k in range(KB[b]):
            gidx = T_A + cumKB[b] + k
            tl.append((int(chunk_of[gidx]), int(slot_of[gidx])))
            tile_block[gidx] = b
        block_tiles.append(tl)

    plan = dict(KA=KA, KB=KB, T=T, T_A=T_A, chunks=chunks,
                block_tiles=block_tiles, tile_block=tile_block,
                n_chunks=n_chunks)

    # ---------------- per-core slot arrays
    dgrid = np.arange(P, dtype=np.int32)[:, None, None]
    per_core = []
    for c in range(NCORES):
        src_rel = np.zeros((T, P), dtype=np.int16)
        dstl = np.full((T, P), PAD_DSTL, dtype=np.float32)
        eww = np.zeros((T, P), dtype=np.float32)
        for b in range(NBLK):
            for half, K, cum, base in ((0, KA, cumKA, 0),
                                       (1, KB, cumKB, T_A)):
                idxs = lists[c][b][half]
                n = idxs.shape[0]
                g0 = base + cum[b]
                nslots = int(K[b]) * P
                s_loc = np.zeros(nslots, dtype=np.int64)
                dl = np.full(nslots, PAD_DSTL, dtype=np.float32)
                w = np.zeros(nslots, dtype=np.float32)
                if n:
                    s_loc[:n] = src_s[idxs] - (HALF if half else 0)
                    dl[:n] = (dst_s[idxs] - (c * NPC + b * P)).astype(
                        np.float32)
                    w[:n] = ew_s[idxs]
                src_rel[g0: g0 + int(K[b])] = \
                    s_loc.reshape(int(K[b]), P).astype(np.int16)
                dstl[g0: g0 + int(K[b])] = dl.reshape(int(K[b]), P)
                eww[g0: g0 + int(K[b])] = w.reshape(int(K[b]), P)

        src_idx = np.zeros((n_chunks, P, IDX_COLS), dtype=np.int16)
        s2s = np.zeros((n_chunks, P, CHUNK_TILES * 256), dtype=np.uint8)
        for ci, ch in enumerate(chunks):
            g0, nt = ch["g0"], ch["nt"]
            src_idx[ci] = _wrap_idx(src_rel[g0: g0 + nt].reshape(nt * P))
            dli = dstl[g0: g0 + nt].astype(np.int32)        # [nt, P]
            eq = (dgrid == dli[None, :, :])                 # [128, nt, P]
            v = s2s[ci].reshape(P, CHUNK_TILES, 256)
            # S[e, t, d] = eq[d, t, e];  S2[d, t, e] = eq[d, t, e]
            v[:, :nt, 0:128] = \
                eq.transpose(2, 1, 0).astype(np.uint8) * ONE_FP8
            v[:, :nt, 128:256] = eq.transpose(0, 1, 2).astype(np.uint8) \
                * ONE_FP8

        per_core.append(dict(src_idx=src_idx, s2s=s2s,
                             ew=np.ascontiguousarray(eww)))

    return plan, per_core


# ---------------------------------------------------------------- builder
def build(plan):
    n_chunks = plan["n_chunks"]
    chunks = plan["chunks"]
    T = plan["T"]
    tile_block = plan["tile_block"]

    nc = bacc.Bacc("TRN2", target_bir_lowering=False, debug=False,
                   num_devices=NCORES, num_swdge_queues=4)
    qctr = [0]

    # inputs
    x_t_full = nc.dram_tensor("x_t_full", [P, NPAD], BF, kind="ExternalInput")
    x_t_own = nc.dram_tensor("x_t_own", [P, NPC], BF, kind="ExternalInput")
    rhsw_in = nc.dram_tensor("rhsw_in", [P, IN_F + 2 * H], BF,
                             kind="ExternalInput")
    biasrep = nc.dram_tensor("biasrep", [P, OUT_F], FP, kind="ExternalInput")
    c_in = nc.dram_tensor("c_in", [P, T * H], BF, kind="ExternalInput")
    s2s_in = nc.dram_tensor("s2s_in", [n_chunks, P, CHUNK_TILES * 256], U8,
                            kind="ExternalInput")
    srcidx_in = nc.dram_tensor("srcidx_in", [n_chunks, P, IDX_COLS],
                               mybir.dt.int16, kind="ExternalInput")
    out = nc.dram_tensor("out", [NPC, OUT_F], FP, kind="ExternalOutput")

    copy_engines = [None, None]  # filled inside (Pool cannot access PSUM)

    with tile.TileContext(nc) as tc:
        for _rep in range(REPS):
            with tc.tile_pool(name="dram", bufs=1, space="DRAM") as dram, \
                 tc.tile_pool(name="statics", bufs=1) as statics:

                hs_t = dram.tile([NPAD, ROWE], BF)

                # ---------------- statics
                bias_sb = statics.tile([P, OUT_F], FP)
                nc.sync.dma_start(bias_sb[:], biasrep[:])
                rhsw_sb = statics.tile([P, IN_F + 2 * H], BF)
                nc.sync.dma_start(rhsw_sb[:], rhsw_in[:])
                c_sb = statics.tile([P, T * H], BF)
                nc.sync.dma_start(c_sb[:], c_in[:])
                sidx_all = statics.tile([P, n_chunks, IDX_COLS],
                                        mybir.dt.int16)
                nc.sync.dma_start(sidx_all[:],
                                  srcidx_in[:].rearrange("c p i -> p c i"))
                ad_sb = statics.tile([P, NBLK * H], BF)

                copy_engines[0] = nc.vector
                copy_engines[1] = nc.scalar

                # ---------------- phase 1: replicated full node table
                with tc.tile_pool(name="p1x", bufs=2) as p1x, \
                     tc.tile_pool(name="p1s", bufs=2) as p1s, \
                     tc.tile_pool(name="p1ps", bufs=4, space="PSUM") as p1ps:

                    eng = [0]

                    def copy_out(dst_ap, src_ap):
                        e = copy_engines[eng[0] % 2]
                        eng[0] += 1
                        if e is nc.scalar:
                            e.activation(dst_ap, src_ap,
                                         mybir.ActivationFunctionType.Copy)
                        else:
                            e.tensor_copy(dst_ap, src_ap)

                    for piece in range(NCORES):
                        xp = p1x.tile([P, NPC], BF, tag="xp")
                        nc.sync.dma_start(
                            xp[:], x_t_full[:, piece * NPC:
                                            (piece + 1) * NPC])
                        stage = p1s.tile([P, NBLK * (IN_F + 2 * H)], BF,
                                         tag="stage")
                        for g0 in range(0, NBLK, GRP):
                            ng = min(GRP, NBLK - g0)
                            ps = p1ps.tile([P, GRP * (IN_F + 2 * H)], FP,
                                           space="PSUM", tag="ps")
                            for j in range(ng):
                                b = g0 + j
                                nc.tensor.matmul(
                                    out=ps[:, j * 144: j * 144 + 144],
                                    lhsT=xp[:, b * P: (b + 1) * P],
                                    rhs=rhsw_sb[:], start=True, stop=True)
                            copy_out(stage[:, g0 * 144: (g0 + ng) * 144],
                                     ps[:, 0: ng * 144])
                        nc.sync.dma_start(
                            hs_t[piece * NPC: (piece + 1) * NPC, 0:136]
                            .rearrange("(t p) r -> p t r", p=P),
                            stage[:].rearrange("p (t r) -> p t r", r=144)
                            [:, :, 8:144])

                    # ad for own node range (rhs = wad = rhsw[:, 0:8])
                    xo = p1x.tile([P, NPC], BF, tag="xo")
                    nc.sync.dma_start(xo[:], x_t_own[:])
                    for g0 in range(0, NBLK, 8):
                        ng = min(8, NBLK - g0)
                        ps = p1ps.tile([P, 64], FP, space="PSUM", tag="adps")
                        for j in range(ng):
                            b = g0 + j
                            nc.tensor.matmul(
                                out=ps[:, j * H: (j + 1) * H],
                                lhsT=xo[:, b * P: (b + 1) * P],
                                rhs=rhsw_sb[:, 0:H], start=True, stop=True)
                        nc.scalar.activation(
                            ad_sb[:, g0 * H: (g0 + ng) * H],
                            ps[:, 0: ng * H],
                            mybir.ActivationFunctionType.Copy)

                # ---------------- phase 2
                with tc.tile_pool(name="gp", bufs=4) as gp, \
                     tc.tile_pool(name="s2p", bufs=4) as s2p, \
                     tc.tile_pool(name="rp", bufs=4) as rp, \
                     tc.tile_pool(name="ep", bufs=4) as ep, \
                     tc.tile_pool(name="op", bufs=3) as opool, \
                     tc.tile_pool(name="adps", bufs=4, space="PSUM") as adps, \
                     tc.tile_pool(name="bps", bufs=4, space="PSUM") as bps:

                    chunk_tiles = {}

                    def emit_chunk(ci):
                        ch = chunks[ci]
                        g0, nt = ch["g0"], ch["nt"]
                        nidx = nt * P
                        n16 = nidx // 16
                        if BUILD_STAGE == 1:
                            return

                        gbuf = gp.tile([P, CHUNK_TILES, ROWE], BF, tag="gbuf")
                        half_ap = (hs_t[0:HALF, :] if ch["stream"] == 0
                                   else hs_t[HALF:NPAD, :])
                        nc.gpsimd.dma_gather(
                            out_ap=gbuf[:, :nt, :], in_ap=half_ap,
                            idxs_ap=sidx_all[:, ci, :n16],
                            num_idxs=nidx, num_idxs_reg=nidx, elem_size=ROWE,
                            single_packet=False, queue_num=qctr[0] % 4)
                        qctr[0] += 1

                        s2b = s2p.tile([P, CHUNK_TILES, 256], U8, tag="s2b")
                        nc.sync.dma_start(s2b[:, :nt, :], s2s_in[ci]
                                          .rearrange("p (t r) -> p t r",
                                                     r=256)[:, :nt, :])
                        if BUILD_STAGE == 2:
                            chunk_tiles[ci] = (s2b, gbuf)
                            return

                        # per-edge ad via S2^T @ ad_blk
                        adp = adps.tile([P, CHUNK_TILES * H], FP,
                                        space="PSUM", tag="adp")
                        for u in range(nt):
                            blk = int(tile_block[g0 + u])
                            nc.tensor.matmul(
                                out=adp[:, u * H: (u + 1) * H],
                                lhsT=s2b[:, u, 128:256].bitcast(FP8),
                                rhs=ad_sb[:, blk * H: (blk + 1) * H],
                                start=True, stop=True)

                        # scores: z = as + ad ; l = lrelu(z) ; e5 = l + c
                        zb = ep.tile([P, CHUNK_TILES * H], BF, tag="zb")
                        zv = zb[:].rearrange("p (t h) -> p t h", h=H)
                        nc.vector.tensor_tensor(
                            out=zv[:, :nt, :],
                            in0=gbuf[:, :nt, AS_OFF: AS_OFF + H],
                            in1=adp[:].rearrange("p (t h) -> p t h", h=H)
                            [:, :nt, :],
                            op=mybir.AluOpType.add)
                        lb = ep.tile([P, CHUNK_TILES * H], BF, tag="lb")
                        nc.vector.scalar_tensor_tensor(
                            out=lb[:, : nt * H], in0=zb[:, : nt * H],
                            scalar=ALPHA, in1=zb[:, : nt * H],
                            op0=mybir.AluOpType.mult,
                            op1=mybir.AluOpType.max)
                        e5 = ep.tile([P, CHUNK_TILES * H], BF, tag="e5")
                        nc.vector.tensor_tensor(
                            out=e5[:, : nt * H], in0=lb[:, : nt * H],
                            in1=c_sb[:, g0 * H: (g0 + nt) * H],
                            op=mybir.AluOpType.add)

                        # rhs tile: [msgs_dmaj(128) | p(8)] per tile
                        rhs = rp.tile([P, CHUNK_TILES * (OUT_F + H)], BF,
                                      tag="rhs")
                        rhs_v = rhs[:].rearrange("p (t f) -> p t f",
                                                 f=OUT_F + H)
                        nc.scalar.activation(
                            rhs_v[:, :nt, OUT_F: OUT_F + H],
                            e5[:].rearrange("p (t h) -> p t h", h=H)
                            [:, :nt, :],
                            mybir.ActivationFunctionType.Exp)
                        # msgs = h * p (d-major h: bcast on d dim, 2x mode)
                        nc.vector.tensor_tensor(
                            out=rhs_v[:, :nt, 0:OUT_F].rearrange(
                                "p t (d h) -> p t d h", h=H),
                            in0=gbuf[:, :nt, 0:OUT_F].rearrange(
                                "p t (d h) -> p t d h", h=H),
                            in1=rhs_v[:, :nt, OUT_F: OUT_F + H].unsqueeze(2)
                                .broadcast_to([P, nt, HD, H]),
                            op=mybir.AluOpType.mult)
                        chunk_tiles[ci] = (s2b, rhs)

                    if BUILD_STAGE < 4:
                        for ci in range(n_chunks):
                            emit_chunk(ci)
                        dump = opool.tile([P, OUT_F], FP, tag="dump")
                        if BUILD_STAGE == 1:
                            nc.vector.memset(dump[:], 0.0)
                        elif BUILD_STAGE == 2:
                            g0buf = chunk_tiles[0][1]      # gbuf [P, CT, ROWE]
                            nc.vector.tensor_copy(dump[:],
                                                  g0buf[:, 0, 0:OUT_F])
                        else:
                            r0 = chunk_tiles[0][1]          # rhs [P, CT*136]
                            nc.vector.tensor_copy(dump[:], r0[:, 0:OUT_F])
                        for b in range(NBLK):
                            nc.sync.dma_start(out[b * P: (b + 1) * P, :],
                                              dump[:])

                    for b in range(NBLK if BUILD_STAGE >= 4 else 0):
                        tl = plan["block_tiles"][b]
                        for (ci, slot) in tl:
                            if ci not in chunk_tiles:
                                emit_chunk(ci)
                        psum_b = bps.tile([P, OUT_F + H], FP, space="PSUM",
                                          tag="psum_b")
                        for i, (ci, slot) in enumerate(tl):
                            s2b, rhs = chunk_tiles[ci]
                            nc.tensor.matmul(
                                out=psum_b[:],
                                lhsT=s2b[:, slot, 0:128].bitcast(FP8),
                                rhs=rhs[:, slot * (OUT_F + H):
                                        (slot + 1) * (OUT_F + H)],
                                start=(i == 0), stop=(i == len(tl) - 1))
                        # normalize + bias; out is h-major
                        s_eps = opool.tile([P, H], FP, tag="s_eps")
                        nc.scalar.activation(
                            s_eps[:], psum_b[:, OUT_F: OUT_F + H],
                            mybir.ActivationFunctionType.Copy, bias=EPS)
                        rcp = opool.tile([P, H], FP, tag="rcp")
                        nc.vector.reciprocal(rcp[:], s_eps[:])
                        ob1 = opool.tile([P, OUT_F], FP, tag="ob1")
                        nc.vector.tensor_tensor(
                            out=ob1[:].rearrange("p (h d) -> p d h", d=HD),
                            in0=psum_b[:, 0:OUT_F].rearrange(
                                "p (d h) -> p d h", h=H),
                            in1=rcp[:].unsqueeze(1).broadcast_to([P, HD, H]),
                            op=mybir.AluOpType.mult)
                        ob2 = opool.tile([P, OUT_F], FP, tag="ob2")
                        nc.gpsimd.tensor_tensor(out=ob2[:], in0=ob1[:],
                                                in1=bias_sb[:],
                                                op=mybir.AluOpType.add)
                        nc.sync.dma_start(out[b * P: (b + 1) * P, :], ob2[:])

    nc.compile()
    # SWDGE constraint: a DMA semaphore may only be updated from one queue.
    # Tile assigns DMASW lanes post-scheduling, so align queue_num to lane.
    for f in nc.m.functions:
        for bb in f.blocks:
            for ins in bb.instructions:
                if type(ins).__name__ == "InstDMAGatherAnt":
                    si = ins.sync_info
                    lane = None
                    for u in si.on_update:
                        nm = u.ant_name or ""
                        if nm.startswith("DMASW"):
                            lane = int(nm[5:].split("_")[0])
                            break
                    assert lane is not None, "gather without DMASW sem"
                    ins.queue_num = lane % 4
    return nc


# ---------------------------------------------------------------- host API
def make_in_maps(x, W, a_src, a_dst, ep_w, ep_b, bias, per_core):
    import ml_dtypes
    bf16 = ml_dtypes.bfloat16
    x = np.asarray(x, dtype=np.float32)
    W = np.asarray(W, dtype=np.float32)
    a_src = np.asarray(a_src, dtype=np.float32)
    a_dst = np.asarray(a_dst, dtype=np.float32)
    ep_w = np.asarray(ep_w, dtype=np.float32)
    ep_b = np.asarray(ep_b, dtype=np.float32)
    bias = np.asarray(bias, dtype=np.float32)

    x_pad = np.zeros((NPAD, IN_F), dtype=np.float32)
    x_pad[:N] = x
    x_t = np.ascontiguousarray(x_pad.T.astype(bf16))         # [128, NPAD]

    # rhs_w = [wad(8) | W_dmaj(128) | was(8)]
    wad = np.einsum("hio,ho->ih", W, a_dst)                  # [IN, H]
    was = np.einsum("hio,ho->ih", W, a_src)                  # [IN, H]
    w_dmaj = np.transpose(W, (1, 2, 0)).reshape(IN_F, HD * H)
    rhsw = np.concatenate([wad, w_dmaj, was], axis=1).astype(bf16)

    rep = lambda v: np.ascontiguousarray(
        np.broadcast_to(v[None, :], (P, v.shape[0])))

    maps = []
    for c in range(NCORES):
        pc = per_core[c]
        # c = ew*epw + epb ; layout [P(slot), T*H]
        cc = pc["ew"][:, :, None] * ep_w[None, None, :] \
            + ep_b[None, None, :]                            # [T, P, H]
        c_t = np.ascontiguousarray(
            cc.transpose(1, 0, 2).reshape(P, -1).astype(bf16))
        x_t_own = np.ascontiguousarray(
            x_pad[c * NPC: (c + 1) * NPC, :].T.astype(bf16))
        maps.append({
            "x_t_full": x_t,
            "x_t_own": x_t_own,
            "rhsw_in": rhsw,
            "biasrep": rep(bias),
            "c_in": c_t,
            "s2s_in": pc["s2s"],
            "srcidx_in": pc["src_idx"],
        })
    return maps


_CACHE = {}


def kernel(x, edge_index, edge_weight, W, a_src, a_dst, ep_w, ep_b, bias):
    import hashlib
    key = hashlib.sha1(
        np.ascontiguousarray(np.asarray(edge_index, dtype=np.int64))
    ).hexdigest()
    if key not in _CACHE:
        plan, per_core = plan_and_inputs(edge_index, edge_weight)
        nc = build(plan)
        _CACHE[key] = (plan, per_core, nc)
    plan, per_core, nc = _CACHE[key]

    in_maps = make_in_maps(x, W, a_src, a_dst, ep_w, ep_b, bias, per_core)
    res = run_bass_kernel_spmd(nc, in_maps, core_ids=list(range(NCORES)),
                               trace=False)
    out_full = np.empty((NPAD, OUT_F), dtype=np.float32)
    for c in range(NCORES):
        out_full[c * NPC: (c + 1) * NPC] = res.results[c]["out"]
    return out_full[:N]


# revision 20
# speedup vs baseline: 1.2017x; 1.2017x over previous
"""EnhancedCorrelationGNN Trainium2 kernel (8 NeuronCores, SPMD), v3.

Strategy: destination-sorted edge processing, node-range output sharding,
bf16 edge pipeline.
 - Host (free): counting-sort edges by dst, partition nodes into 8 ranges of
   6272 (49 blocks x 128 nodes per core). Per block, edges are split by src
   half (dma_gather int16 index limit) and padded to 128-edge tiles with
   cross-core-uniform tile counts (one SPMD program). Host also ships, per
   chunk, the one-hot scatter matrices S (edge->dst, agg lhsT) and S2
   (dst->edge, ad-apply lhsT) as fp8 bytes, plus c = ew*epw+epb in bf16.
 - Phase 1 (device): each core computes h|as|ad for its OWN node range in
   ONE matmul per 128-node tile (rhs = [W@a_dst | W_dmaj | W@a_src] bf16),
   writes 512B bf16 rows [h_dmaj(128)|as(8)|pad] and AllGathers the table.
   ad rows stay in SBUF (block-local dst).
 - Phase 2 (device): per 32-tile chunk, ONE dma_gather of 512B bf16 rows by
   src (a separate <512B ad gather would cost as much DMA time - hence the
   S2-matmul instead), one u8 DMA of [S|S2]; per tile a tiny matmul
   S2^T@ad_blk gives per-edge ad; scores via DVE add -> leaky(stt) ->
   add c -> ScalarE Exp (p bf16); msgs = h*p on DVE 2x (d-major h layout);
   per-tile agg matmul (lhsT=S fp8, rhs=[msgs|p] bf16) accumulates
   [sum msgs | sum p] per block. Chunks are prefetched ahead of the block
   loop so gather latency stays off the critical path; finalize
   (1/(sum p+eps), bias) is batched 4 blocks at a time.
"""
import sys

if "/opt/trn_rl_repo" not in sys.path:
    sys.path.insert(0, "/opt/trn_rl_repo")

import numpy as np

import concourse.bass as bass
import concourse.bacc as bacc
import concourse.mybir as mybir
import concourse.tile as tile
from concourse.bass_utils import run_bass_kernel_spmd

# ---------------------------------------------------------------- constants
N = 50000
E = 800000
IN_F = 128
H = 8
HD = 16
OUT_F = H * HD          # 128
ALPHA = 0.2
EPS = 1e-10

NCORES = 8
P = 128
NPC = 6272              # nodes per core = 49 * 128; 8*6272 = 50176 >= N
NPAD = NCORES * NPC     # 50176
NBLK = NPC // P         # 49
HALF = NPAD // 2        # 25088 (int16 gather index limit)

ROWE = 256              # hs table row: bf16 elems [h_dmaj(128)|as(8)|pad]
AS_OFF = 128
CHUNK_TILES = 32        # tiles per gather/DVE chunk
IDX_COLS = CHUNK_TILES * P // 16   # wrapped int16 idx columns per chunk
PAD_DSTL = 300.0        # one-hot miss sentinel
GRP = 3                 # phase-1 matmul tiles batched per PSUM copy
PREFETCH = 2            # chunks emitted ahead of block consumption
FINB = 4                # blocks finalized per batch

FP = mybir.dt.float32
BF = mybir.dt.bfloat16
U8 = mybir.dt.uint8
FP8 = mybir.dt.float8e4
ONE_FP8 = 0x38          # 1.0 in e4m3

# build stages for HW bisection: 1=phase1+AG, 2=+gathers+S loads,
# 3=+chunk compute pipeline, 4=full (default)
BUILD_STAGE = 4
# repeat whole kernel body inside one NEFF (for timing by differencing)
REPS = 1


# ---------------------------------------------------------------- planning
def _cdiv(a, b):
    return -(-a // b)


def _wrap_idx(idx_flat: np.ndarray) -> np.ndarray:
    """[n] -> [128, IDX_COLS] int16: idx j at [j%16, j//16], replicated x8."""
    n = idx_flat.shape[0]
    assert n % 16 == 0
    w = idx_flat.reshape(n // 16, 16).T.astype(np.int16)      # [16, n/16]
    w = np.tile(w, (8, 1))                                    # [128, n/16]
    out = np.zeros((P, IDX_COLS), dtype=np.int16)
    out[:, : w.shape[1]] = w
    return out


def plan_and_inputs(edge_index, edge_weight):
    """Host-side edge partitioning. Returns (plan, per_core_arrays)."""
    src = np.asarray(edge_index[0], dtype=np.int64)
    dst = np.asarray(edge_index[1], dtype=np.int64)
    ew = np.asarray(edge_weight, dtype=np.float32)

    order = np.argsort(dst, kind="stable")
    src_s, dst_s, ew_s = src[order], dst[order], ew[order]

    cnt = np.zeros((NCORES, NBLK, 2), dtype=np.int64)
    lists = [[[None, None] for _ in range(NBLK)] for _ in range(NCORES)]
    blk_starts = np.searchsorted(dst_s, np.arange(0, NPAD + 1, P))
    for c in range(NCORES):
        for b in range(NBLK):
            g = c * NBLK + b
            lo, hi = blk_starts[g], blk_starts[g + 1]
            s = src_s[lo:hi]
            mA = s < HALF
            idxs = np.arange(lo, hi)
            lists[c][b][0] = idxs[mA]
            lists[c][b][1] = idxs[~mA]
            cnt[c, b, 0] = mA.sum()
            cnt[c, b, 1] = (~mA).sum()

    KA = np.maximum(_cdiv(cnt[:, :, 0].max(axis=0), P), 1).astype(np.int64)
    KB = _cdiv(cnt[:, :, 1].max(axis=0), P).astype(np.int64)

    T_A = int(KA.sum())
    T_B = int(KB.sum())
    T = T_A + T_B
    cumKA = np.concatenate([[0], np.cumsum(KA)])
    cumKB = np.concatenate([[0], np.cumsum(KB)])

    chunks = []
    g = 0
    while g < T_A:
        nt = min(CHUNK_TILES, T_A - g)
        chunks.append(dict(stream=0, g0=g, nt=nt))
        g += nt
    while g < T:
        nt = min(CHUNK_TILES, T - g)
        chunks.append(dict(stream=1, g0=g, nt=nt))
        g += nt
    n_chunks = len(chunks)

    chunk_of = np.empty(T, dtype=np.int64)
    slot_of = np.empty(T, dtype=np.int64)
    for ci, ch in enumerate(chunks):
        chunk_of[ch["g0"]: ch["g0"] + ch["nt"]] = ci
        slot_of[ch["g0"]: ch["g0"] + ch["nt"]] = np.arange(ch["nt"])

    tile_block = np.empty(T, dtype=np.int64)
    block_tiles = []
    for b in range(NBLK):
        tl = []
        for k in range(KA[b]):
            gidx = cumKA[b] + k
            tl.append((int(chunk_of[gidx]), int(slot_of[gidx])))
            tile_block[gidx] = b
        for k in range(KB[b]):
            gidx = T_A + cumKB[b] + k
            tl.append((int(chunk_of[gidx]), int(slot_of[gidx])))
            tile_block[gidx] = b
        block_tiles.append(tl)

    plan = dict(KA=KA, KB=KB, T=T, T_A=T_A, chunks=chunks,
                block_tiles=block_tiles, tile_block=tile_block,
                n_chunks=n_chunks)

    # ---------------- per-core slot arrays
    dgrid = np.arange(P, dtype=np.int32)[:, None, None]
    per_core = []
    for c in range(NCORES):
        src_rel = np.zeros((T, P), dtype=np.int16)
        dstl = np.full((T, P), PAD_DSTL, dtype=np.float32)
        eww = np.zeros((T, P), dtype=np.float32)
        for b in range(NBLK):
            for half, K, cum, base in ((0, KA, cumKA, 0),
                                       (1, KB, cumKB, T_A)):
                idxs = lists[c][b][half]
                n = idxs.shape[0]
                g0 = base + cum[b]
                nslots = int(K[b]) * P
                s_loc = np.zeros(nslots, dtype=np.int64)
                dl = np.full(nslots, PAD_DSTL, dtype=np.float32)
                w = np.zeros(nslots, dtype=np.float32)
                if n:
                    s_loc[:n] = src_s[idxs] - (HALF if half else 0)
                    dl[:n] = (dst_s[idxs] - (c * NPC + b * P)).astype(
                        np.float32)
                    w[:n] = ew_s[idxs]
                src_rel[g0: g0 + int(K[b])] = \
                    s_loc.reshape(int(K[b]), P).astype(np.int16)
                dstl[g0: g0 + int(K[b])] = dl.reshape(int(K[b]), P)
                eww[g0: g0 + int(K[b])] = w.reshape(int(K[b]), P)

        src_idx = np.zeros((n_chunks, P, IDX_COLS), dtype=np.int16)
        s_oh = np.zeros((n_chunks, P, CHUNK_TILES * 128), dtype=np.uint8)
        s2_oh = np.zeros((n_chunks, P, CHUNK_TILES * 128), dtype=np.uint8)
        for ci, ch in enumerate(chunks):
            g0, nt = ch["g0"], ch["nt"]
            src_idx[ci] = _wrap_idx(src_rel[g0: g0 + nt].reshape(nt * P))
            dli = dstl[g0: g0 + nt].astype(np.int32)        # [nt, P]
            eq = (dgrid == dli[None, :, :])                 # [128, nt, P]
            # S[e, t, d] = eq[d, t, e];  S2[d, t, e] = eq[d, t, e]
            s_oh[ci].reshape(P, CHUNK_TILES, 128)[:, :nt, :] = \
                eq.transpose(2, 1, 0).astype(np.uint8) * ONE_FP8
            s2_oh[ci].reshape(P, CHUNK_TILES, 128)[:, :nt, :] = \
                eq.astype(np.uint8) * ONE_FP8

        per_core.append(dict(src_idx=src_idx, s_oh=s_oh, s2_oh=s2_oh,
                             ew=np.ascontiguousarray(eww)))

    return plan, per_core


# ---------------------------------------------------------------- builder
def build(plan):
    n_chunks = plan["n_chunks"]
    chunks = plan["chunks"]
    T = plan["T"]
    tile_block = plan["tile_block"]
    block_tiles = plan["block_tiles"]

    nc = bacc.Bacc("TRN2", target_bir_lowering=False, debug=False,
                   num_devices=NCORES, num_swdge_queues=4)
    qctr = [0]

    # inputs
    x_t_own = nc.dram_tensor("x_t_own", [P, NPC], BF, kind="ExternalInput")
    rhsw_in = nc.dram_tensor("rhsw_in", [P, IN_F + 2 * H], BF,
                             kind="ExternalInput")
    biasrep = nc.dram_tensor("biasrep", [P, OUT_F], FP, kind="ExternalInput")
    c_in = nc.dram_tensor("c_in", [P, T * H], BF, kind="ExternalInput")
    s_in = nc.dram_tensor("s_in", [n_chunks, P, CHUNK_TILES * 128], U8,
                          kind="ExternalInput")
    s2_in = nc.dram_tensor("s2_in", [n_chunks, P, CHUNK_TILES * 128], U8,
                           kind="ExternalInput")
    srcidx_in = nc.dram_tensor("srcidx_in", [n_chunks, P, IDX_COLS],
                               mybir.dt.int16, kind="ExternalInput")
    out = nc.dram_tensor("out", [NPC, OUT_F], FP, kind="ExternalOutput")

    with tile.TileContext(nc) as tc:
        for _rep in range(REPS):
            with tc.tile_pool(name="dram", bufs=1, space="DRAM") as dram, \
                 tc.tile_pool(name="statics", bufs=1) as statics:

                hs_in = dram.tile([NPC, ROWE], BF)
                hs_full = dram.tile([NPAD, ROWE], BF, addr_space="Shared")

                # ---------------- statics
                bias_sb = statics.tile([P, OUT_F], FP)
                nc.sync.dma_start(bias_sb[:], biasrep[:])
                rhsw_sb = statics.tile([P, IN_F + 2 * H], BF)
                nc.sync.dma_start(rhsw_sb[:], rhsw_in[:])
                c_sb = statics.tile([P, T * H], BF)
                nc.sync.dma_start(c_sb[:], c_in[:])
                sidx_all = statics.tile([P, n_chunks, IDX_COLS],
                                        mybir.dt.int16)
                nc.sync.dma_start(sidx_all[:],
                                  srcidx_in[:].rearrange("c p i -> p c i"))
                ad_sb = statics.tile([P, NBLK * H], BF)

                # ---------------- phase 1: own node range + AllGather
                with tc.tile_pool(name="p1x", bufs=1) as p1x, \
                     tc.tile_pool(name="p1s", bufs=1) as p1s, \
                     tc.tile_pool(name="p1ps", bufs=4, space="PSUM") as p1ps:

                    xo = p1x.tile([P, NPC], BF)
                    nc.sync.dma_start(xo[:], x_t_own[:])
                    stage = p1s.tile([P, NBLK * (IN_F + 2 * H)], BF)
                    eng = [0]
                    for g0 in range(0, NBLK, GRP):
                        ng = min(GRP, NBLK - g0)
                        ps = p1ps.tile([P, GRP * 144], FP, space="PSUM",
                                       tag="ps")
                        for j in range(ng):
                            b = g0 + j
                            nc.tensor.matmul(
                                out=ps[:, j * 144: j * 144 + 144],
                                lhsT=xo[:, b * P: (b + 1) * P],
                                rhs=rhsw_sb[:], start=True, stop=True)
                        if eng[0] % 2 == 0:
                            nc.vector.tensor_copy(
                                stage[:, g0 * 144: (g0 + ng) * 144],
                                ps[:, 0: ng * 144])
                        else:
                            nc.scalar.activation(
                                stage[:, g0 * 144: (g0 + ng) * 144],
                                ps[:, 0: ng * 144],
                                mybir.ActivationFunctionType.Copy)
                        eng[0] += 1
                    # ad rows (block-local dst) stay in SBUF
                    nc.gpsimd.tensor_copy(
                        ad_sb[:].rearrange("p (t h) -> p t h", h=H),
                        stage[:].rearrange("p (t r) -> p t r", r=144)
                        [:, :, 0:H])
                    nc.sync.dma_start(
                        hs_in[:, 0:136].rearrange("(t p) r -> p t r", p=P),
                        stage[:].rearrange("p (t r) -> p t r", r=144)
                        [:, :, 8:144])

                nc.gpsimd.collective_compute(
                    "AllGather", mybir.AluOpType.bypass,
                    replica_groups=[list(range(NCORES))],
                    ins=[hs_in[:]], outs=[hs_full[:]],
                )

                # ---------------- phase 2
                # gp/s2p/ep: consumed within emit_chunk (short-lived);
                # sp/rp: consumed by the block loop (long-lived window)
                with tc.tile_pool(name="gp", bufs=3) as gp, \
                     tc.tile_pool(name="s2p", bufs=3) as s2p, \
                     tc.tile_pool(name="sp", bufs=6) as sp, \
                     tc.tile_pool(name="rp", bufs=6) as rp, \
                     tc.tile_pool(name="ep", bufs=3) as ep, \
                     tc.tile_pool(name="fin", bufs=2) as finp, \
                     tc.tile_pool(name="adps", bufs=2, space="PSUM") as adps, \
                     tc.tile_pool(name="bps", bufs=6, space="PSUM") as bps:

                    chunk_tiles = {}

                    def emit_chunk(ci):
                        if ci in chunk_tiles or ci >= n_chunks:
                            return
                        ch = chunks[ci]
                        g0, nt = ch["g0"], ch["nt"]
                        nidx = nt * P
                        n16 = nidx // 16
                        if BUILD_STAGE == 1:
                            return

                        gbuf = gp.tile([P, CHUNK_TILES, ROWE], BF, tag="gbuf")
                        half_ap = (hs_full[0:HALF, :] if ch["stream"] == 0
                                   else hs_full[HALF:NPAD, :])
                        nc.gpsimd.dma_gather(
                            out_ap=gbuf[:, :nt, :], in_ap=half_ap,
                            idxs_ap=sidx_all[:, ci, :n16],
                            num_idxs=nidx, num_idxs_reg=nidx, elem_size=ROWE,
                            single_packet=False, queue_num=qctr[0] % 4)
                        qctr[0] += 1

                        s_b = sp.tile([P, CHUNK_TILES, 128], U8, tag="s_b")
                        nc.sync.dma_start(s_b[:, :nt, :], s_in[ci]
                                          .rearrange("p (t r) -> p t r",
                                                     r=128)[:, :nt, :])
                        s2b = s2p.tile([P, CHUNK_TILES, 128], U8, tag="s2b")
                        nc.sync.dma_start(s2b[:, :nt, :], s2_in[ci]
                                          .rearrange("p (t r) -> p t r",
                                                     r=128)[:, :nt, :])
                        if BUILD_STAGE == 2:
                            chunk_tiles[ci] = (s_b, gbuf)
                            return

                        # per-edge ad via S2^T @ ad_blk
                        adp = adps.tile([P, CHUNK_TILES * H], FP,
                                        space="PSUM", tag="adp")
                        for u in range(nt):
                            blk = int(tile_block[g0 + u])
                            nc.tensor.matmul(
                                out=adp[:, u * H: (u + 1) * H],
                                lhsT=s2b[:, u, :].bitcast(FP8),
                                rhs=ad_sb[:, blk * H: (blk + 1) * H],
                                start=True, stop=True)

                        # scores: z = as + ad ; l = lrelu(z) ; e5 = l + c
                        zb = ep.tile([P, CHUNK_TILES * H], BF, tag="zb")
                        zv = zb[:].rearrange("p (t h) -> p t h", h=H)
                        nc.vector.tensor_tensor(
                            out=zv[:, :nt, :],
                            in0=gbuf[:, :nt, AS_OFF: AS_OFF + H],
                            in1=adp[:].rearrange("p (t h) -> p t h", h=H)
                            [:, :nt, :],
                            op=mybir.AluOpType.add)
                        lb = ep.tile([P, CHUNK_TILES * H], BF, tag="lb")
                        nc.vector.scalar_tensor_tensor(
                            out=lb[:, : nt * H], in0=zb[:, : nt * H],
                            scalar=ALPHA, in1=zb[:, : nt * H],
                            op0=mybir.AluOpType.mult,
                            op1=mybir.AluOpType.max)
                        e5 = ep.tile([P, CHUNK_TILES * H], BF, tag="e5")
                        nc.vector.tensor_tensor(
                            out=e5[:, : nt * H], in0=lb[:, : nt * H],
                            in1=c_sb[:, g0 * H: (g0 + nt) * H],
                            op=mybir.AluOpType.add)

                        # rhs tile: [msgs_dmaj(128) | p(8)] per tile
                        rhs = rp.tile([P, CHUNK_TILES * (OUT_F + H)], BF,
                                      tag="rhs")
                        rhs_v = rhs[:].rearrange("p (t f) -> p t f",
                                                 f=OUT_F + H)
                        nc.scalar.activation(
                            rhs_v[:, :nt, OUT_F: OUT_F + H],
                            e5[:].rearrange("p (t h) -> p t h", h=H)
                            [:, :nt, :],
                            mybir.ActivationFunctionType.Exp)
                        # msgs = h * p (d-major h: bcast on d dim, 2x mode)
                        nc.vector.tensor_tensor(
                            out=rhs_v[:, :nt, 0:OUT_F].rearrange(
                                "p t (d h) -> p t d h", h=H),
                            in0=gbuf[:, :nt, 0:OUT_F].rearrange(
                                "p t (d h) -> p t d h", h=H),
                            in1=rhs_v[:, :nt, OUT_F: OUT_F + H].unsqueeze(2)
                                .broadcast_to([P, nt, HD, H]),
                            op=mybir.AluOpType.mult)
                        chunk_tiles[ci] = (s_b, rhs)

                    if BUILD_STAGE < 4:
                        for ci in range(n_chunks):
                            emit_chunk(ci)
                        dump = finp.tile([P, OUT_F], FP, tag="dump")
                        if BUILD_STAGE == 1:
                            nc.vector.memset(dump[:], 0.0)
                        elif BUILD_STAGE == 2:
                            g0buf = chunk_tiles[0][1]      # gbuf [P, CT, ROWE]
                            nc.vector.tensor_copy(dump[:],
                                                  g0buf[:, 0, 0:OUT_F])
                        else:
                            r0 = chunk_tiles[0][1]          # rhs [P, CT*136]
                            nc.vector.tensor_copy(dump[:], r0[:, 0:OUT_F])
                        for b in range(NBLK):
                            nc.sync.dma_start(out[b * P: (b + 1) * P, :],
                                              dump[:])

                    # block loop with chunk prefetch + batched finalize
                    if BUILD_STAGE >= 4:
                        # chunks needed per block, in id order
                        blk_chunks = [sorted({ci for ci, _ in block_tiles[b]})
                                      for b in range(NBLK)]
                        group_psums = []

                        def finalize(group):
                            nb = len(group)
                            b0 = group[0][0]
                            fin = finp.tile([P, FINB * (OUT_F + H)], FP,
                                            tag="fin")
                            fv = fin[:].rearrange("p (k f) -> p k f",
                                                  f=OUT_F + H)
                            for k, (_b, psum_b) in enumerate(group):
                                nc.scalar.activation(
                                    fv[:, k, :], psum_b[:],
                                    mybir.ActivationFunctionType.Copy)
                            se = finp.tile([P, FINB * H], FP, tag="se")
                            nc.vector.tensor_scalar_add(
                                out=se[:, : nb * H]
                                .rearrange("p (k h) -> p k h", h=H),
                                in0=fv[:, :nb, OUT_F: OUT_F + H],
                                scalar1=EPS)
                            rcp = finp.tile([P, FINB * H], FP, tag="rcp")
                            nc.vector.reciprocal(rcp[:, : nb * H],
                                                 se[:, : nb * H])
                            ob = finp.tile([P, FINB * OUT_F], FP, tag="ob")
                            obv = ob[:].rearrange(
                                "p (k h d) -> p k d h", h=H, d=HD)
                            nc.vector.tensor_tensor(
                                out=obv[:, :nb],
                                in0=fv[:, :nb, 0:OUT_F].rearrange(
                                    "p k (d h) -> p k d h", h=H),
                                in1=rcp[:, : nb * H].rearrange(
                                    "p (k h) -> p k h", h=H).unsqueeze(2)
                                    .broadcast_to([P, nb, HD, H]),
                                op=mybir.AluOpType.mult)
                            ob2 = finp.tile([P, FINB * OUT_F], FP, tag="ob2")
                            nc.gpsimd.tensor_tensor(
                                out=ob2[:, : nb * OUT_F]
                                .rearrange("p (k f) -> p k f", f=OUT_F),
                                in0=ob[:, : nb * OUT_F]
                                .rearrange("p (k f) -> p k f", f=OUT_F),
                                in1=bias_sb[:].unsqueeze(1)
                                .broadcast_to([P, nb, OUT_F]),
                                op=mybir.AluOpType.add)
                            nc.sync.dma_start(
                                out[b0 * P: (b0 + nb) * P, :]
                                .rearrange("(k p) f -> p k f", p=P),
                                ob2[:, : nb * OUT_F]
                                .rearrange("p (k f) -> p k f", f=OUT_F))

                        for b in range(NBLK):
                            # prefetch chunks for blocks [b, b+PREFETCH]
                            for w in range(b, min(b + PREFETCH,
                                                  NBLK - 1) + 1):
                                for ci in blk_chunks[w]:
                                    emit_chunk(ci)
                            tl = block_tiles[b]
                            psum_b = bps.tile([P, OUT_F + H], FP,
                                              space="PSUM", tag="psum_b")
                            for i, (ci, slot) in enumerate(tl):
                                s_b, rhs = chunk_tiles[ci]
                                nc.tensor.matmul(
                                    out=psum_b[:],
                                    lhsT=s_b[:, slot, :].bitcast(FP8),
                                    rhs=rhs[:, slot * (OUT_F + H):
                                            (slot + 1) * (OUT_F + H)],
                                    start=(i == 0), stop=(i == len(tl) - 1))
                            group_psums.append((b, psum_b))
                            if len(group_psums) == FINB or b == NBLK - 1:
                                finalize(group_psums)
                                group_psums = []

    nc.compile()
    # SWDGE constraint: a DMA semaphore may only be updated from one queue.
    # Tile assigns DMASW lanes post-scheduling, so align queue_num to lane.
    for f in nc.m.functions:
        for bb in f.blocks:
            for ins in bb.instructions:
                if type(ins).__name__ == "InstDMAGatherAnt":
                    si = ins.sync_info
                    lane = None
                    for u in si.on_update:
                        nm = u.ant_name or ""
                        if nm.startswith("DMASW"):
                            lane = int(nm[5:].split("_")[0])
                            break
                    assert lane is not None, "gather without DMASW sem"
                    ins.queue_num = lane % 4
    return nc


# ---------------------------------------------------------------- host API
def make_in_maps(x, W, a_src, a_dst, ep_w, ep_b, bias, per_core):
    import ml_dtypes
    bf16 = ml_dtypes.bfloat16
    x = np.asarray(x, dtype=np.float32)
    W = np.asarray(W, dtype=np.float32)
    a_src = np.asarray(a_src, dtype=np.float32)
    a_dst = np.asarray(a_dst, dtype=np.float32)
    ep_w = np.asarray(ep_w, dtype=np.float32)
    ep_b = np.asarray(ep_b, dtype=np.float32)
    bias = np.asarray(bias, dtype=np.float32)

    x_pad = np.zeros((NPAD, IN_F), dtype=np.float32)
    x_pad[:N] = x

    # rhs_w = [wad(8) | W_dmaj(128) | was(8)]
    wad = np.einsum("hio,ho->ih", W, a_dst)                  # [IN, H]
    was = np.einsum("hio,ho->ih", W, a_src)                  # [IN, H]
    w_dmaj = np.transpose(W, (1, 2, 0)).reshape(IN_F, HD * H)
    rhsw = np.ascontiguousarray(
        np.concatenate([wad, w_dmaj, was], axis=1).astype(bf16))

    rep = lambda v: np.ascontiguousarray(
        np.broadcast_to(v[None, :], (P, v.shape[0])))

    maps = []
    for c in range(NCORES):
        pc = per_core[c]
        cc = pc["ew"][:, :, None] * ep_w[None, None, :] \
            + ep_b[None, None, :]                            # [T, P, H]
        c_t = np.ascontiguousarray(
            cc.transpose(1, 0, 2).reshape(P, -1).astype(bf16))
        x_t_own = np.ascontiguousarray(
            x_pad[c * NPC: (c + 1) * NPC, :].T.astype(bf16))
        maps.append({
            "x_t_own": x_t_own,
            "rhsw_in": rhsw,
            "biasrep": rep(bias),
            "c_in": c_t,
            "s_in": pc["s_oh"],
            "s2_in": pc["s2_oh"],
            "srcidx_in": pc["src_idx"],
        })
    return maps


_CACHE = {}


def kernel(x, edge_index, edge_weight, W, a_src, a_dst, ep_w, ep_b, bias):
    import hashlib
    key = hashlib.sha1(
        np.ascontiguousarray(np.asarray(edge_index, dtype=np.int64))
    ).hexdigest()
    if key not in _CACHE:
        plan, per_core = plan_and_inputs(edge_index, edge_weight)
        nc = build(plan)
        _CACHE[key] = (plan, per_core, nc)
    plan, per_core, nc = _CACHE[key]

    in_maps = make_in_maps(x, W, a_src, a_dst, ep_w, ep_b, bias, per_core)
    res = run_bass_kernel_spmd(nc, in_maps, core_ids=list(range(NCORES)),
                               trace=False)
    out_full = np.empty((NPAD, OUT_F), dtype=np.float32)
    for c in range(NCORES):
        out_full[c * NPC: (c + 1) * NPC] = res.results[c]["out"]
    return out_full[:N]
